# revision 1
# baseline (speedup 1.0000x reference)
"""Fused multi-head attention for Trainium2 (Bass/Tile), 8-core SPMD.

Problem: B=2, H=16, S=4096, D=64, fp32, mask == all-ones (unmasked softmax).

Strategy (per core, 4 of the 32 (b,h) heads):
  * S^T orientation flash attention: keys on partitions, queries on the free
    dim, so no on-chip transposes are needed anywhere.
  * QK^T: lhsT = K^T tile [64, 128] (fp32r), rhs = Q^T block [64, 512]
    (fp32r) -> S^T psum tile [128 keys, 512 queries]. K=64 contraction runs
    in the PE's 64-row tiling mode; even key-tiles use array rows 0-63, odd
    key-tiles rows 64-127, so pairs execute concurrently.
  * exp on ScalarE straight out of PSUM in 1536-wide chunks (scale=1/8
    folded into the activation), writing fp32 P^T chunk tiles to SBUF.
    The exp chain is the bottleneck (~494us busy per core) and runs
    gapless; everything else hides underneath it.
  * P@V: V is pre-augmented host-side with a ones column (V' = [V, 1]) so
    the 65th output row accumulates the softmax denominator for free.
    Each 128-key tile is split into two 64-key halves on rows 0-63/64-127
    (again concurrent 64-row-mode pairs) accumulating into two PSUM banks;
    a DVE copy+add merges them. P@V of a q-block trails its exp chain by
    two chunks, spilling into the next q-block, so the PE work interleaves
    between exp chunks instead of bursting.
  * Normalization (divide by denominator) and the final [D, S] -> [S, D]
    transpose happen host-side on the gathered outputs.

Inputs are pre-rearranged host-side (numpy) into the layouts the kernel
wants: Q^T duplicated onto both partition halves, K^T even/odd-packed, and
V' key-tile-major. Input loads use SWDGE (gpsimd) dmas: large HWDGE loads
showed completion-semaphore races against pool-slot reuse on hardware.
"""

import numpy as np

import concourse.mybir as mybir
import concourse.tile as tile
from concourse import bacc
from concourse.bass_utils import run_bass_kernel_spmd

B, H, S, D = 2, 16, 4096, 64
BH = B * H
N_CORES = 8
NH = BH // N_CORES          # heads per core
QB = 512                    # queries per q-block
N_QB = S // QB              # q-blocks per head
KT = S // 128               # 128-key tiles per head
CHUNK = 3                   # key-tiles per exp chunk (3 psum banks)

F32 = mybir.dt.float32
F32R = mybir.dt.float32r

_cache = {}


def _build_program():
    nc = bacc.Bacc(num_swdge_queues=4)
    kt_in = nc.declare_dram_parameter("kt", [NH, 128, S // 2], F32R, isOutput=False)
    qt_in = nc.declare_dram_parameter("qt", [NH, 128, S], F32R, isOutput=False)
    v_in = nc.declare_dram_parameter("v", [NH, 128, KT * 65], F32R, isOutput=False)
    o_out = nc.declare_dram_parameter("o", [NH, 65, S], F32, isOutput=True)

    with tile.TileContext(nc) as tc:
        with (
            tc.tile_pool(name="kt_p", bufs=2) as kt_pool,
            tc.tile_pool(name="qt_p", bufs=2) as qt_pool,
            tc.tile_pool(name="v_p", bufs=2) as v_pool,
            tc.tile_pool(name="pt_p", bufs=8) as pt_pool,
            tc.tile_pool(name="osum_p", bufs=2) as osum_pool,
            tc.tile_pool(name="stage_p", bufs=2, space="PSUM") as stage_pool,
            tc.tile_pool(name="ot_p", bufs=2, space="PSUM") as ot_pool,
        ):
            class PVState:
                """Previous q-block's P@V, emitted chunk-by-chunk between
                the exp chunks so the PE never bursts long enough to starve
                ScalarE. P^T arrives as per-chunk fp32 tiles."""

                def __init__(self, v_s, h, qb):
                    self.v_s, self.h, self.qb = v_s, h, qb
                    self.k = 0
                    self.queue = []
                    self.ot_a = ot_pool.tile([128, QB], F32, tag="ot")
                    self.ot_b = ot_pool.tile([128, QB], F32, tag="ot")

                def add_chunk(self, pt, csz):
                    self.queue.append((pt, csz))

                def emit_chunk(self):
                    pt, csz = self.queue.pop(0)
                    for i in range(csz):
                        k = self.k + i
                        for half, ot in ((0, self.ot_a), (1, self.ot_b)):
                            lhsT = self.v_s[64 * half:64 * half + 64,
                                            k * 65:(k + 1) * 65]
                            rhs = pt[64 * half:64 * half + 64,
                                     i * QB:(i + 1) * QB]
                            nc.tensor.matmul(
                                ot[0:65, :], lhsT, rhs,
                                start=(k == 0), stop=(k == KT - 1),
                                skip_group_check=True,
                            )
                    self.k += csz

                def finish(self):
                    while self.queue:
                        self.emit_chunk()
                    assert self.k == KT
                    osum = osum_pool.tile([128, QB], F32, tag="osum")
                    nc.vector.tensor_copy(osum[0:65, :], self.ot_a[0:65, :])
                    nc.vector.tensor_add(
                        osum[0:65, :], osum[0:65, :], self.ot_b[0:65, :]
                    )
                    nc.sync.dma_start(
                        o_out[self.h, :, self.qb * QB:(self.qb + 1) * QB],
                        osum[0:65, :],
                    )

            def chunked_load(dst, src, widths):
                c0 = 0
                for w in widths:
                    nc.gpsimd.dma_start(dst[:, c0:c0 + w], src[:, c0:c0 + w])
                    c0 += w
                assert c0 == dst.shape[-1]

            chunk_sizes = [CHUNK] * (KT // CHUNK) + (
                [KT % CHUNK] if KT % CHUNK else []
            )

            prev = None    # PV of previous q-block: last 2 chunks + flush left
            cur = None     # PV of current q-block, trailing the exp by 2 chunks
            for h in range(NH):
                # Loads in strict need-order, with the pieces gating the very
                # first QK matmuls split down to partition-half granularity so
                # the exp chain starts as early as possible (matters for h=0;
                # harmless for later heads, whose loads hide under compute).
                kt_s = kt_pool.tile([128, S // 2], F32R, tag="kt")
                qt_s = qt_pool.tile([128, S], F32R, tag="qt")
                v_s = v_pool.tile([128, KT * 65], F32R, tag="v")
                ld = nc.gpsimd.dma_start
                ld(kt_s[0:64, 0:128], kt_in[h][0:64, 0:128])        # key tile 0
                ld(qt_s[0:64, 0:256], qt_in[h][0:64, 0:256])
                ld(qt_s[0:64, 256:QB], qt_in[h][0:64, 256:QB])
                ld(kt_s[64:128, 0:128], kt_in[h][64:128, 0:128])    # key tile 1
                ld(kt_s[0:64, 128:256], kt_in[h][0:64, 128:256])    # key tile 2
                ld(qt_s[64:128, 0:256], qt_in[h][64:128, 0:256])
                ld(qt_s[64:128, 256:QB], qt_in[h][64:128, 256:QB])
                ld(kt_s[64:128, 128:256], kt_in[h][64:128, 128:256])
                # K^T pieces paced to the exp chain's ~130 cols/us consumption
                chunked_load(
                    kt_s[:, 256:S // 2], kt_in[h][:, 256:S // 2], [256] * 7
                )
                chunked_load(v_s[:, :], v_in[h][:, :], [520, 520, 520, 520])
                chunked_load(qt_s[:, QB:2 * QB], qt_in[h][:, QB:2 * QB], [256, 256])
                chunked_load(
                    qt_s[:, 2 * QB:S], qt_in[h][:, 2 * QB:S], [QB] * 6
                )

                for qb in range(N_QB):
                    cur = PVState(v_s, h, qb)
                    col = 0
                    # Very first q-block: two single-tile chunks so the exp
                    # chain fires as soon as key-tile 0 + the top Q^T half
                    # land, ~5us before a 3-tile chunk could.
                    sizes = (
                        [1, 1] + [CHUNK] * 10 if h == 0 and qb == 0
                        else chunk_sizes
                    )
                    for c, csz in enumerate(sizes):
                        st = stage_pool.tile([128, csz * QB], F32, tag="stage")
                        for i in range(csz):
                            k = col + i
                            half = k % 2
                            blk = k // 2
                            lhsT = kt_s[64 * half:64 * half + 64,
                                        blk * 128:(blk + 1) * 128]
                            rhs = qt_s[64 * half:64 * half + 64,
                                       qb * QB:(qb + 1) * QB]
                            nc.tensor.matmul(
                                st[:, i * QB:(i + 1) * QB], lhsT, rhs,
                                start=True, stop=True,
                            )
                        pt = pt_pool.tile([128, csz * QB], F32R, tag="pt")
                        nc.scalar.activation(
                            pt[:, :],
                            st[:, :csz * QB],
                            mybir.ActivationFunctionType.Exp,
                            scale=1.0 / np.sqrt(float(D)),
                        )
                        cur.add_chunk(pt, csz)
                        col += csz
                        # PE filler between exp chunks: drain the previous
                        # q-block's PV leftovers first, then this q-block's
                        # PV trailing two chunks behind the exp chain.
                        if c == 0:
                            if prev is not None:
                                prev.emit_chunk()
                        elif c == 1:
                            if prev is not None:
                                prev.finish()
                                prev = None
                        else:
                            cur.emit_chunk()
                    prev = cur
            prev.finish()

    nc.compile()
    return nc


def _get_program():
    if "nc" not in _cache:
        _cache["nc"] = _build_program()
    return _cache["nc"]


def _pack_inputs(Q, K, V):
    """Host-side rearrangement into per-core device layouts."""
    Qf = np.ascontiguousarray(Q.reshape(BH, S, D))
    Kf = np.ascontiguousarray(K.reshape(BH, S, D))
    Vf = np.ascontiguousarray(V.reshape(BH, S, D))

    # Q^T [BH, 64, S], duplicated onto both partition halves -> [BH, 128, S]
    QT = Qf.transpose(0, 2, 1)
    QTd = np.ascontiguousarray(np.concatenate([QT, QT], axis=1), dtype=np.float32)

    # K^T [BH, 64, S] -> even key-tiles on partitions 0-63, odd on 64-127
    KTm = Kf.transpose(0, 2, 1).reshape(BH, D, KT, 128)
    KTpack = np.concatenate(
        [
            KTm[:, :, 0::2, :].reshape(BH, D, S // 2),
            KTm[:, :, 1::2, :].reshape(BH, D, S // 2),
        ],
        axis=1,
    ).astype(np.float32)

    # V' = [V, ones]; key-tile-major bf16 layout [BH, 128, KT*65]
    Vp = np.concatenate([Vf, np.ones((BH, S, 1), np.float32)], axis=-1)
    Vb = np.ascontiguousarray(
        Vp.reshape(BH, KT, 128, 65)
        .transpose(0, 2, 1, 3)
        .reshape(BH, 128, KT * 65)
    )
    return KTpack, QTd, Vb


def kernel(Q, K, V, mask):
    assert Q.shape == (B, H, S, D)
    nc = _get_program()
    KTpack, QTd, Vb = _pack_inputs(
        np.asarray(Q, dtype=np.float32),
        np.asarray(K, dtype=np.float32),
        np.asarray(V, dtype=np.float32),
    )
    in_maps = []
    for c in range(N_CORES):
        sl = slice(c * NH, (c + 1) * NH)
        in_maps.append(
            {
                "kt": np.ascontiguousarray(KTpack[sl]),
                "qt": np.ascontiguousarray(QTd[sl]),
                "v": np.ascontiguousarray(Vb[sl]),
            }
        )
    res = run_bass_kernel_spmd(nc, in_maps, core_ids=list(range(N_CORES)))
    O = np.concatenate([r["o"] for r in res.results], axis=0)  # [BH, 65, S]
    out = (O[:, :D, :] / O[:, D:D + 1, :]).transpose(0, 2, 1)  # [BH, S, D]
    return np.ascontiguousarray(out.reshape(B, H, S, D).astype(np.float32))



# revision 2
# speedup vs baseline: 1.1098x; 1.1098x over previous
"""Fused multi-head attention for Trainium2 (Bass/Tile), 8-core SPMD.

Problem: B=2, H=16, S=4096, D=64, fp32 in/out, mask == all-ones.

Strategy (per core, 4 of the 32 (b,h) heads):
  * S^T orientation flash attention: keys on partitions, queries on the free
    dim, so no on-chip transposes are needed anywhere.
  * All matmul operands are bf16 (fp32 matmuls cost 2x on both LDWEIGHTS
    and MATMUL streaming; bf16 halves PE time and HBM traffic).
  * QK^T: lhsT = K^T tile [64, 128] bf16, rhs = Q^T block [64, 512] bf16
    -> S^T psum fp32 [128 keys, 512 queries]. Even key-tiles use PE rows
    0-63, odd rows 64-127, so pairs execute concurrently.
  * exp is split across TWO engines working concurrently on alternating
    2-k-tile chunks (1024 cols):
      - ScalarE: native activation Exp (scale=1/8 folded in), bf16 out.
      - DVE: Schraudolph fast-exp in one tensor_scalar: i16 = round(
        s*A + B) where A = 2^7*log2(e)/8, B = 2^7*(127-C).  The int16
        bit pattern IS the bf16 exp approximation (verified round-to-
        nearest on HW); the tile is bitcast to bf16 for P@V.
        End-to-end softmax error from the approximation ~1.3e-2 rel.
  * P@V: V' = [V, 1] (ones column accumulates the softmax denominator);
    full 128-key contraction per key tile, accumulated over all 32 key
    tiles into one PSUM bank, trailing the exp chain by two chunks.
  * Normalization (divide by denominator row) and the final [65, S] ->
    [S, D] transpose happen host-side on the gathered outputs.

Inputs are pre-rearranged host-side (numpy) into the layouts the kernel
wants: Q^T duplicated onto both partition halves, K^T even/odd-packed, and
V' key-tile-major, all bf16. Input loads use SWDGE (gpsimd) dmas: large
HWDGE loads showed completion-semaphore races against pool-slot reuse.
"""

import numpy as np
import ml_dtypes

import concourse.mybir as mybir
import concourse.tile as tile
from concourse import bacc
from concourse.bass_utils import run_bass_kernel_spmd

B, H, S, D = 2, 16, 4096, 64
BH = B * H
N_CORES = 8
NH = BH // N_CORES          # heads per core
QB = 512                    # queries per q-block
N_QB = S // QB              # q-blocks per head
KT = S // 128               # 128-key tiles per head
CHUNK = 2                   # key-tiles per exp chunk (2 psum banks)
N_CH = KT // CHUNK          # chunks per q-block

F32 = mybir.dt.float32
BF16 = mybir.dt.bfloat16
I16 = mybir.dt.int16
NPBF16 = np.dtype(ml_dtypes.bfloat16)

# Schraudolph fast-exp constants (bf16 bit domain), 1/sqrt(D) folded in.
SCHRAUDOLPH_C = 0.0579
A_DVE = 128.0 / (np.log(2.0) * np.sqrt(float(D)))
B_DVE = 128.0 * (127.0 - SCHRAUDOLPH_C)

# ScalarE handles SCALAR_NUM of every SCALAR_DEN chunks (rest on DVE).
SCALAR_NUM, SCALAR_DEN = 17, 32

_cache = {}


def _build_program():
    nc = bacc.Bacc(num_swdge_queues=4)
    kt_in = nc.declare_dram_parameter("kt", [NH, 128, S // 2], BF16, isOutput=False)
    qt_in = nc.declare_dram_parameter("qt", [NH, 128, S], BF16, isOutput=False)
    v_in = nc.declare_dram_parameter("v", [NH, 128, KT * 65], BF16, isOutput=False)
    o_out = nc.declare_dram_parameter("o", [NH, 65, S], F32, isOutput=True)

    with tile.TileContext(nc) as tc:
        with (
            tc.tile_pool(name="kt_p", bufs=2) as kt_pool,
            tc.tile_pool(name="qt_p", bufs=2) as qt_pool,
            tc.tile_pool(name="v_p", bufs=2) as v_pool,
            tc.tile_pool(name="pts_p", bufs=4) as pts_pool,
            tc.tile_pool(name="ptd_p", bufs=4) as ptd_pool,
            tc.tile_pool(name="osum_p", bufs=2) as osum_pool,
            tc.tile_pool(name="stage_p", bufs=3, space="PSUM") as stage_pool,
            tc.tile_pool(name="ot_p", bufs=2, space="PSUM") as ot_pool,
        ):
            class PVState:
                """P@V for one q-block, emitted chunk-by-chunk between the
                exp chunks so the PE interleaves QK / PV work instead of
                bursting. Full 128-key contraction per key tile into one
                PSUM accumulator; the 65th row is the softmax denominator
                (ones column of V')."""

                def __init__(self, v_s, h, qb):
                    self.v_s, self.h, self.qb = v_s, h, qb
                    self.k = 0
                    self.queue = []
                    self.ot = ot_pool.tile([65, QB], F32, tag="ot")

                def add_chunk(self, ptv, csz):
                    self.queue.append((ptv, csz))

                def emit_chunk(self):
                    ptv, csz = self.queue.pop(0)
                    for i in range(csz):
                        k = self.k + i
                        nc.tensor.matmul(
                            self.ot[:, :],
                            self.v_s[:, k * 65:(k + 1) * 65],
                            ptv[:, i * QB:(i + 1) * QB],
                            start=(k == 0), stop=(k == KT - 1),
                            skip_group_check=True,
                        )
                    self.k += csz

                def finish(self):
                    while self.queue:
                        self.emit_chunk()
                    assert self.k == KT
                    osum = osum_pool.tile([65, QB], F32, tag="osum")
                    nc.scalar.copy(osum[:, :], self.ot[:, :])
                    nc.sync.dma_start(
                        o_out[self.h, :, self.qb * QB:(self.qb + 1) * QB],
                        osum[:, :],
                    )

            def chunked_load(dst, src, widths):
                c0 = 0
                for w in widths:
                    nc.gpsimd.dma_start(dst[:, c0:c0 + w], src[:, c0:c0 + w])
                    c0 += w
                assert c0 == dst.shape[-1]

            prev = None    # PV of previous q-block: last 2 chunks + flush left
            cur = None     # PV of current q-block, trailing the exp by 2 chunks
            g = 0          # global chunk counter (engine assignment)
            for h in range(NH):
                # Loads in strict need-order; the pieces gating the very
                # first QK matmuls are split small so the exp chain starts
                # as early as possible (matters for h=0).
                kt_s = kt_pool.tile([128, S // 2], BF16, tag="kt")
                qt_s = qt_pool.tile([128, S], BF16, tag="qt")
                v_s = v_pool.tile([128, KT * 65], BF16, tag="v")
                ld = nc.gpsimd.dma_start
                ld(kt_s[0:64, 0:128], kt_in[h][0:64, 0:128])        # key tile 0
                ld(kt_s[64:128, 0:128], kt_in[h][64:128, 0:128])    # key tile 1
                ld(qt_s[0:64, 0:QB], qt_in[h][0:64, 0:QB])
                ld(qt_s[64:128, 0:QB], qt_in[h][64:128, 0:QB])
                ld(kt_s[0:64, 128:256], kt_in[h][0:64, 128:256])    # tiles 2,3
                ld(kt_s[64:128, 128:256], kt_in[h][64:128, 128:256])
                # K^T remainder paced in pieces
                chunked_load(
                    kt_s[:, 256:S // 2], kt_in[h][:, 256:S // 2], [448] * 4
                )
                chunked_load(v_s[:, :], v_in[h][:, :], [520, 520, 520, 520])
                chunked_load(qt_s[:, QB:2 * QB], qt_in[h][:, QB:2 * QB], [256, 256])
                chunked_load(
                    qt_s[:, 2 * QB:S], qt_in[h][:, 2 * QB:S], [QB] * 6
                )

                for qb in range(N_QB):
                    cur = PVState(v_s, h, qb)
                    for c in range(N_CH):
                        st = stage_pool.tile([128, CHUNK * QB], F32, tag="stage")
                        for i in range(CHUNK):
                            k = c * CHUNK + i
                            half = k % 2
                            blk = k // 2
                            lhsT = kt_s[64 * half:64 * half + 64,
                                        blk * 128:(blk + 1) * 128]
                            rhs = qt_s[64 * half:64 * half + 64,
                                       qb * QB:(qb + 1) * QB]
                            nc.tensor.matmul(
                                st[:, i * QB:(i + 1) * QB], lhsT, rhs,
                                start=True, stop=True,
                            )
                        use_scalar = (
                            ((g + 1) * SCALAR_NUM) // SCALAR_DEN
                            != (g * SCALAR_NUM) // SCALAR_DEN
                        )
                        g += 1
                        if use_scalar:
                            pt = pts_pool.tile([128, CHUNK * QB], BF16, tag="pts")
                            nc.scalar.activation(
                                pt[:, :],
                                st[:, :],
                                mybir.ActivationFunctionType.Exp,
                                scale=1.0 / np.sqrt(float(D)),
                            )
                            ptv = pt[:, :]
                        else:
                            pt = ptd_pool.tile([128, CHUNK * QB], I16, tag="ptd")
                            nc.vector.tensor_scalar(
                                pt[:, :], st[:, :],
                                float(A_DVE), float(B_DVE),
                                mybir.AluOpType.mult, mybir.AluOpType.add,
                            )
                            ptv = pt[:, :].bitcast(BF16)
                        cur.add_chunk(ptv, CHUNK)
                        # PE filler between exp chunks: drain the previous
                        # q-block's PV leftovers first, then this q-block's
                        # PV trailing two chunks behind the exp chain.
                        if c == 0:
                            if prev is not None:
                                prev.emit_chunk()
                        elif c == 1:
                            if prev is not None:
                                prev.finish()
                                prev = None
                        else:
                            cur.emit_chunk()
                    prev = cur
            prev.finish()

    nc.compile()
    return nc


def _get_program():
    if "nc" not in _cache:
        _cache["nc"] = _build_program()
    return _cache["nc"]


def _pack_inputs(Q, K, V):
    """Host-side rearrangement into per-core device layouts (bf16)."""
    Qf = np.ascontiguousarray(Q.reshape(BH, S, D))
    Kf = np.ascontiguousarray(K.reshape(BH, S, D))
    Vf = np.ascontiguousarray(V.reshape(BH, S, D))

    # Q^T [BH, 64, S], duplicated onto both partition halves -> [BH, 128, S]
    QT = Qf.transpose(0, 2, 1)
    QTd = np.ascontiguousarray(
        np.concatenate([QT, QT], axis=1).astype(NPBF16)
    )

    # K^T [BH, 64, S] -> even key-tiles on partitions 0-63, odd on 64-127
    KTm = Kf.transpose(0, 2, 1).reshape(BH, D, KT, 128)
    KTpack = np.concatenate(
        [
            KTm[:, :, 0::2, :].reshape(BH, D, S // 2),
            KTm[:, :, 1::2, :].reshape(BH, D, S // 2),
        ],
        axis=1,
    ).astype(NPBF16)

    # V' = [V, ones]; key-tile-major layout [BH, 128, KT*65]
    Vp = np.concatenate([Vf, np.ones((BH, S, 1), np.float32)], axis=-1)
    Vb = np.ascontiguousarray(
        Vp.reshape(BH, KT, 128, 65)
        .transpose(0, 2, 1, 3)
        .reshape(BH, 128, KT * 65)
        .astype(NPBF16)
    )
    return KTpack, QTd, Vb


def kernel(Q, K, V, mask):
    assert Q.shape == (B, H, S, D)
    nc = _get_program()
    KTpack, QTd, Vb = _pack_inputs(
        np.asarray(Q, dtype=np.float32),
        np.asarray(K, dtype=np.float32),
        np.asarray(V, dtype=np.float32),
    )
    in_maps = []
    for c in range(N_CORES):
        sl = slice(c * NH, (c + 1) * NH)
        in_maps.append(
            {
                "kt": np.ascontiguousarray(KTpack[sl]),
                "qt": np.ascontiguousarray(QTd[sl]),
                "v": np.ascontiguousarray(Vb[sl]),
            }
        )
    res = run_bass_kernel_spmd(nc, in_maps, core_ids=list(range(N_CORES)))
    O = np.concatenate([r["o"] for r in res.results], axis=0)  # [BH, 65, S]
    out = (O[:, :D, :] / O[:, D:D + 1, :]).transpose(0, 2, 1)  # [BH, S, D]
    return np.ascontiguousarray(out.reshape(B, H, S, D).astype(np.float32))


# revision 3
# speedup vs baseline: 1.2067x; 1.0873x over previous
"""Fused multi-head attention for Trainium2 (Bass/Tile), 8-core SPMD.

Problem: B=2, H=16, S=4096, D=64, fp32 in/out, mask == all-ones.

Strategy (per core, 4 of the 32 (b,h) heads):
  * S^T orientation flash attention: keys on partitions, queries on the free
    dim, so no on-chip transposes are needed anywhere.
  * All matmul operands are bf16 (fp32 matmuls cost 2x on both LDWEIGHTS
    and MATMUL streaming; bf16 halves PE time and HBM traffic).
  * QK^T: lhsT = K^T tile [64, 128] bf16, rhs = Q^T block [64, 512] bf16
    -> S^T psum fp32 [128 keys, 512 queries]. Even key-tiles use PE rows
    0-63, odd rows 64-127, so pairs execute concurrently.
  * exp is split across TWO engines working concurrently on alternating
    2-k-tile chunks (1024 cols):
      - ScalarE: native activation Exp (scale=1/8 folded in), bf16 out.
      - DVE: Schraudolph fast-exp in one tensor_scalar: i16 = round(
        s*A + B) where A = 2^7*log2(e)/8, B = 2^7*(127-C).  The int16
        bit pattern IS the bf16 exp approximation (verified round-to-
        nearest on HW); the tile is bitcast to bf16 for P@V.
        End-to-end softmax error from the approximation ~1.3e-2 rel.
  * P@V: V' = [V, 1] (ones column accumulates the softmax denominator);
    full 128-key contraction per key tile, accumulated over all 32 key
    tiles into one PSUM bank, trailing the exp chain by two chunks.
  * Normalization (divide by denominator row) and the final [65, S] ->
    [S, D] transpose happen host-side on the gathered outputs.

Inputs are pre-rearranged host-side (numpy) into the layouts the kernel
wants: Q^T duplicated onto both partition halves, K^T even/odd-packed, and
V' key-tile-major, all bf16. Input loads use SWDGE (gpsimd) dmas: large
HWDGE loads showed completion-semaphore races against pool-slot reuse.
"""

import numpy as np
import ml_dtypes

import concourse.mybir as mybir
import concourse.tile as tile
from concourse import bacc
from concourse.bass_utils import run_bass_kernel_spmd

B, H, S, D = 2, 16, 4096, 64
BH = B * H
N_CORES = 8
NH = BH // N_CORES          # heads per core
QB = 512                    # queries per q-block
N_QB = S // QB              # q-blocks per head
KT = S // 128               # 128-key tiles per head
CHUNK = 2                   # key-tiles per exp chunk (2 psum banks)
N_CH = KT // CHUNK          # chunks per q-block

F32 = mybir.dt.float32
BF16 = mybir.dt.bfloat16
I16 = mybir.dt.int16
NPBF16 = np.dtype(ml_dtypes.bfloat16)

# Schraudolph fast-exp constants (bf16 bit domain), 1/sqrt(D) folded in.
SCHRAUDOLPH_C = 0.0579
A_DVE = 128.0 / (np.log(2.0) * np.sqrt(float(D)))
B_DVE = 128.0 * (127.0 - SCHRAUDOLPH_C)

# ScalarE handles SCALAR_NUM of every SCALAR_DEN chunks (rest on DVE).
SCALAR_NUM, SCALAR_DEN = 17, 32

_cache = {}


def _build_program():
    nc = bacc.Bacc(num_swdge_queues=4)
    kt_in = nc.declare_dram_parameter("kt", [NH, 128, S // 2], BF16, isOutput=False)
    qt_in = nc.declare_dram_parameter("qt", [NH, 128, S], BF16, isOutput=False)
    v_in = nc.declare_dram_parameter("v", [NH, 128, KT * 65], BF16, isOutput=False)
    o_out = nc.declare_dram_parameter("o", [NH, 65, S], F32, isOutput=True)

    with tile.TileContext(nc) as tc:
        with (
            tc.tile_pool(name="kt_p", bufs=2) as kt_pool,
            tc.tile_pool(name="qt_p", bufs=2) as qt_pool,
            tc.tile_pool(name="v_p", bufs=2) as v_pool,
            tc.tile_pool(name="pts_p", bufs=4) as pts_pool,
            tc.tile_pool(name="ptd_p", bufs=4) as ptd_pool,
            tc.tile_pool(name="osum_p", bufs=2) as osum_pool,
            tc.tile_pool(name="stage_p", bufs=3, space="PSUM") as stage_pool,
            tc.tile_pool(name="ot_p", bufs=2, space="PSUM") as ot_pool,
        ):
            class PVState:
                """P@V for one q-block, emitted chunk-by-chunk between the
                exp chunks so the PE interleaves QK / PV work instead of
                bursting. Each 128-key tile is split into two 64-key halves
                on PE rows 0-63 / 64-127 running CONCURRENTLY into two PSUM
                banks (full-row matmuls would block LDWEIGHTS preloading of
                the next matmul and expose ~100ns of drain per boundary).
                ScalarE copy + DVE add merge the banks; the 65th row is the
                softmax denominator (ones column of V')."""

                def __init__(self, v_s, h, qb):
                    self.v_s, self.h, self.qb = v_s, h, qb
                    self.k = 0
                    self.queue = []
                    self.ot_a = ot_pool.tile([65, QB], F32, tag="ot")
                    self.ot_b = ot_pool.tile([65, QB], F32, tag="ot")

                def add_chunk(self, ptv, csz):
                    self.queue.append((ptv, csz))

                def emit_chunk(self):
                    ptv, csz = self.queue.pop(0)
                    for i in range(csz):
                        k = self.k + i
                        for half, ot in ((0, self.ot_a), (1, self.ot_b)):
                            nc.tensor.matmul(
                                ot[:, :],
                                self.v_s[64 * half:64 * half + 64,
                                         k * 65:(k + 1) * 65],
                                ptv[64 * half:64 * half + 64,
                                    i * QB:(i + 1) * QB],
                                start=(k == 0), stop=(k == KT - 1),
                                skip_group_check=True,
                            )
                    self.k += csz

                def finish(self):
                    while self.queue:
                        self.emit_chunk()
                    assert self.k == KT
                    osum = osum_pool.tile([65, QB], F32, tag="osum")
                    nc.scalar.copy(osum[:, :], self.ot_a[:, :])
                    nc.vector.tensor_add(osum[:, :], osum[:, :], self.ot_b[:, :])
                    nc.sync.dma_start(
                        o_out[self.h, :, self.qb * QB:(self.qb + 1) * QB],
                        osum[:, :],
                    )

            def chunked_load(dst, src, widths):
                c0 = 0
                for w in widths:
                    nc.gpsimd.dma_start(dst[:, c0:c0 + w], src[:, c0:c0 + w])
                    c0 += w
                assert c0 == dst.shape[-1]

            prev = None    # PV of previous q-block: last 2 chunks + flush left
            cur = None     # PV of current q-block, trailing the exp by 2 chunks
            g = 0          # global chunk counter (engine assignment)
            for h in range(NH):
                # Loads in strict need-order; the pieces gating the very
                # first QK matmuls are split small so the exp chain starts
                # as early as possible (matters for h=0).
                kt_s = kt_pool.tile([128, S // 2], BF16, tag="kt")
                qt_s = qt_pool.tile([128, S], BF16, tag="qt")
                v_s = v_pool.tile([128, KT * 65], BF16, tag="v")
                ld = nc.gpsimd.dma_start
                ld(kt_s[0:64, 0:128], kt_in[h][0:64, 0:128])        # key tile 0
                ld(kt_s[64:128, 0:128], kt_in[h][64:128, 0:128])    # key tile 1
                ld(qt_s[0:64, 0:QB], qt_in[h][0:64, 0:QB])
                ld(qt_s[64:128, 0:QB], qt_in[h][64:128, 0:QB])
                ld(kt_s[0:64, 128:256], kt_in[h][0:64, 128:256])    # tiles 2,3
                ld(kt_s[64:128, 128:256], kt_in[h][64:128, 128:256])
                # K^T remainder paced in pieces
                chunked_load(
                    kt_s[:, 256:S // 2], kt_in[h][:, 256:S // 2], [448] * 4
                )
                chunked_load(v_s[:, :], v_in[h][:, :], [520, 520, 520, 520])
                chunked_load(qt_s[:, QB:2 * QB], qt_in[h][:, QB:2 * QB], [256, 256])
                chunked_load(
                    qt_s[:, 2 * QB:S], qt_in[h][:, 2 * QB:S], [QB] * 6
                )

                for qb in range(N_QB):
                    cur = PVState(v_s, h, qb)
                    for c in range(N_CH):
                        st = stage_pool.tile([128, CHUNK * QB], F32, tag="stage")
                        for i in range(CHUNK):
                            k = c * CHUNK + i
                            half = k % 2
                            blk = k // 2
                            lhsT = kt_s[64 * half:64 * half + 64,
                                        blk * 128:(blk + 1) * 128]
                            rhs = qt_s[64 * half:64 * half + 64,
                                       qb * QB:(qb + 1) * QB]
                            nc.tensor.matmul(
                                st[:, i * QB:(i + 1) * QB], lhsT, rhs,
                                start=True, stop=True,
                            )
                        use_scalar = (
                            ((g + 1) * SCALAR_NUM) // SCALAR_DEN
                            != (g * SCALAR_NUM) // SCALAR_DEN
                        )
                        g += 1
                        if use_scalar:
                            pt = pts_pool.tile([128, CHUNK * QB], BF16, tag="pts")
                            nc.scalar.activation(
                                pt[:, :],
                                st[:, :],
                                mybir.ActivationFunctionType.Exp,
                                scale=1.0 / np.sqrt(float(D)),
                            )
                            ptv = pt[:, :]
                        else:
                            pt = ptd_pool.tile([128, CHUNK * QB], I16, tag="ptd")
                            nc.vector.tensor_scalar(
                                pt[:, :], st[:, :],
                                float(A_DVE), float(B_DVE),
                                mybir.AluOpType.mult, mybir.AluOpType.add,
                            )
                            ptv = pt[:, :].bitcast(BF16)
                        cur.add_chunk(ptv, CHUNK)
                        # PE filler between exp chunks: drain the previous
                        # q-block's PV leftovers first, then this q-block's
                        # PV trailing two chunks behind the exp chain.
                        if c == 0:
                            if prev is not None:
                                prev.emit_chunk()
                        elif c == 1:
                            if prev is not None:
                                prev.finish()
                                prev = None
                        else:
                            cur.emit_chunk()
                    prev = cur
            prev.finish()

    nc.compile()
    return nc


def _get_program():
    if "nc" not in _cache:
        _cache["nc"] = _build_program()
    return _cache["nc"]


def _pack_inputs(Q, K, V):
    """Host-side rearrangement into per-core device layouts (bf16)."""
    Qf = np.ascontiguousarray(Q.reshape(BH, S, D))
    Kf = np.ascontiguousarray(K.reshape(BH, S, D))
    Vf = np.ascontiguousarray(V.reshape(BH, S, D))

    # Q^T [BH, 64, S], duplicated onto both partition halves -> [BH, 128, S]
    QT = Qf.transpose(0, 2, 1)
    QTd = np.ascontiguousarray(
        np.concatenate([QT, QT], axis=1).astype(NPBF16)
    )

    # K^T [BH, 64, S] -> even key-tiles on partitions 0-63, odd on 64-127
    KTm = Kf.transpose(0, 2, 1).reshape(BH, D, KT, 128)
    KTpack = np.concatenate(
        [
            KTm[:, :, 0::2, :].reshape(BH, D, S // 2),
            KTm[:, :, 1::2, :].reshape(BH, D, S // 2),
        ],
        axis=1,
    ).astype(NPBF16)

    # V' = [V, ones]; key-tile-major layout [BH, 128, KT*65]
    Vp = np.concatenate([Vf, np.ones((BH, S, 1), np.float32)], axis=-1)
    Vb = np.ascontiguousarray(
        Vp.reshape(BH, KT, 128, 65)
        .transpose(0, 2, 1, 3)
        .reshape(BH, 128, KT * 65)
        .astype(NPBF16)
    )
    return KTpack, QTd, Vb


def kernel(Q, K, V, mask):
    assert Q.shape == (B, H, S, D)
    nc = _get_program()
    KTpack, QTd, Vb = _pack_inputs(
        np.asarray(Q, dtype=np.float32),
        np.asarray(K, dtype=np.float32),
        np.asarray(V, dtype=np.float32),
    )
    in_maps = []
    for c in range(N_CORES):
        sl = slice(c * NH, (c + 1) * NH)
        in_maps.append(
            {
                "kt": np.ascontiguousarray(KTpack[sl]),
                "qt": np.ascontiguousarray(QTd[sl]),
                "v": np.ascontiguousarray(Vb[sl]),
            }
        )
    res = run_bass_kernel_spmd(nc, in_maps, core_ids=list(range(N_CORES)))
    O = np.concatenate([r["o"] for r in res.results], axis=0)  # [BH, 65, S]
    out = (O[:, :D, :] / O[:, D:D + 1, :]).transpose(0, 2, 1)  # [BH, S, D]
    return np.ascontiguousarray(out.reshape(B, H, S, D).astype(np.float32))
